# revision 3
# baseline (speedup 1.0000x reference)
"""GNN message-passing kernel for Trainium2 (8 NeuronCores, SPMD).

Computes out[D] = mean_n relu(segment_sum(val * (feat @ W.T + b)[src], dst))
reformulated as:
    agg[n]  = sum_{e: dst=n} val[e] * feature[src[e]]      (dma_gather + PE one-hot scatter)
    s[n]    = sum_{e: dst=n} val[e]                        (host-precomputed)
    z[n]    = agg[n] @ W.T + s[n] * b                      (PE, bias as K=1 rank-1 matmul)
    out     = sum_n relu(z[n]) / N                         (PE ones-reduce, host-combined)

Sharding: edges partitioned by destination node across 8 cores. Each core owns
N/8 destination nodes, split into groups of 128. Source-feature rows are
fetched with dma_gather (int16 indices => the node table is processed in
windows of 32768 rows). Per (window, group) bucket, 128-edge blocks are
scattered into a PSUM accumulator via one-hot matmuls; window partials
accumulate into an SBUF agg array. A per-group tail applies W/bias/relu and
reduces into a [1, D] partial that the host sums across cores.
"""

import math
import os
import sys

import numpy as np

LAST_EXEC_NS = None

for _p in ("/opt/trn_rl_repo",):
    if _p not in sys.path:
        sys.path.insert(0, _p)

import concourse.bacc as bacc
import concourse.mybir as mybir
import concourse.tile as tile
from concourse.bass_utils import run_bass_kernel_spmd

P = 128
N_CORES = 8
WS = 32768  # gather window rows (int16 index reach)

F32 = None  # set at import below
try:
    F32 = mybir.dt.float32
except Exception:  # pragma: no cover
    pass


def _plan(N, E, edge_src, edge_dst, edge_val):
    """Host-side layout planning. Returns per-core input arrays + the
    compile-time block structure (shared across cores)."""
    nodes_per_core = (N + N_CORES - 1) // N_CORES
    n_groups = (nodes_per_core + P - 1) // P
    n_win = (N + WS - 1) // WS

    core = edge_dst // nodes_per_core
    local = edge_dst - core * nodes_per_core
    g = local // P
    ld = local % P
    w = edge_src // WS
    src_rel = (edge_src - w * WS).astype(np.int64)

    # bucket = (core, w, g)
    bkey = (core * n_win + w) * n_groups + g
    n_buckets = N_CORES * n_win * n_groups
    counts = np.bincount(bkey, minlength=n_buckets).reshape(N_CORES, n_win, n_groups)
    nb = np.ceil(counts.max(axis=0) / P).astype(np.int64)  # [n_win, n_groups]

    # window processing order: largest edge-count window last
    wtot = counts.sum(axis=(0, 2))
    worder = list(np.argsort(wtot, kind="stable"))

    # window-local block starts per bucket, window sizes
    b0 = np.zeros((n_win, n_groups), np.int64)
    NW = np.zeros(n_win, np.int64)
    for wi in range(n_win):
        b0[wi] = np.cumsum(nb[wi]) - nb[wi]
        NW[wi] = nb[wi].sum()
    woff = np.zeros(n_win, np.int64)  # global block offset, in processing order
    acc = 0
    for wv in worder:
        woff[wv] = acc
        acc += NW[wv]
    B = int(acc)

    # per-edge placement
    order = np.argsort(bkey, kind="stable")
    flat_counts = counts.reshape(-1)
    bucket_start = np.cumsum(flat_counts) - flat_counts
    rank = np.arange(E, dtype=np.int64) - bucket_start[bkey[order]]
    w_s = w[order]
    g_s = g[order]
    c_s = core[order]
    ipos = b0[w_s, g_s] * P + rank  # window-local gather position
    lane = ipos % P
    wblk = ipos // P
    gblk = woff[w_s] + wblk

    ldv = np.zeros((N_CORES, P, B), np.float32)
    valv = np.zeros((N_CORES, P, B), np.float32)
    ldv[c_s, lane, gblk] = ld[order].astype(np.float32)
    valv[c_s, lane, gblk] = edge_val[order]

    idx_wins = []
    for wv in range(n_win):
        iw = np.zeros((N_CORES, 16, int(NW[wv]) * 8), np.int16)
        idx_wins.append(iw)
    sel_w = [w_s == wv for wv in range(n_win)]
    for wv in range(n_win):
        m = sel_w[wv]
        ip = ipos[m]
        idx_wins[wv][c_s[m], ip % 16, ip // 16] = src_rel[order][m].astype(np.int16)
    idx_full = [np.tile(iw, (1, 8, 1)) for iw in idx_wins]  # [NC, 128, NW*8]

    skey = (core * n_groups + g) * P + ld
    s = np.bincount(
        skey, weights=edge_val.astype(np.float64), minlength=N_CORES * n_groups * P
    ).astype(np.float32)
    s = s.reshape(N_CORES, 1, n_groups * P)

    # compile-time structure: per window (processing order) bucket list
    structure = []
    for wv in worder:
        buckets = [(int(gg), int(nb[wv, gg])) for gg in range(n_groups) if nb[wv, gg] > 0]
        structure.append(
            {
                "w": int(wv),
                "row0": int(wv * WS),
                "rows": int(min(WS, N - wv * WS)),
                "NW": int(NW[wv]),
                "woff": int(woff[wv]),
                "buckets": buckets,
            }
        )
    return {
        "nodes_per_core": nodes_per_core,
        "n_groups": n_groups,
        "n_win": n_win,
        "B": B,
        "structure": structure,
        "ldv": ldv,
        "valv": valv,
        "idx": idx_full,
        "s": s,
    }


def _build_program(N, D, plan, dt, oh_pool_frac=0.0, CH=8):
    f32 = mybir.dt.float32
    n_groups = plan["n_groups"]
    B = plan["B"]
    structure = plan["structure"]
    nc = bacc.Bacc("TRN2", target_bir_lowering=False, debug=False, num_devices=N_CORES)

    feature_t = nc.dram_tensor("feature", [N, D], dt, kind="ExternalInput")
    ld_t = nc.dram_tensor("ld", [P, B], dt, kind="ExternalInput")
    val_t = nc.dram_tensor("val", [P, B], dt, kind="ExternalInput")
    s_t = nc.dram_tensor("s", [1, n_groups * P], f32, kind="ExternalInput")
    wt_t = nc.dram_tensor("wt", [D, D], f32, kind="ExternalInput")
    brow_t = nc.dram_tensor("brow", [1, D], f32, kind="ExternalInput")
    iota_t = nc.dram_tensor("iota", [P, P], dt, kind="ExternalInput")
    ident_t = nc.dram_tensor("ident", [P, P], f32, kind="ExternalInput")
    ones_t = nc.dram_tensor("ones", [P, 1], f32, kind="ExternalInput")
    idx_ts = [
        nc.dram_tensor(f"idxw{st['w']}", [P, st["NW"] * 8], mybir.dt.int16,
                       kind="ExternalInput")
        for st in structure
    ]
    out_t = nc.dram_tensor("out", [1, D], f32, kind="ExternalOutput")

    # first/last window (processing order) in which each group has blocks
    last_win_of_g = {}
    first_win_of_g = {}
    seen_g = set()
    for si, st in enumerate(structure):
        for gg, _ in st["buckets"]:
            last_win_of_g[gg] = si
            if gg not in first_win_of_g:
                first_win_of_g[gg] = si
            seen_g.add(gg)

    with tile.TileContext(nc) as tc:
        with (
            tc.tile_pool(name="const", bufs=1) as constp,
            tc.tile_pool(name="idxp", bufs=2) as idxp,
            tc.tile_pool(name="msg", bufs=3) as msgp,
            tc.tile_pool(name="oh", bufs=8) as ohp,
            tc.tile_pool(name="gsb", bufs=3) as gsbp,
            tc.tile_pool(name="agg", bufs=2, space="PSUM") as aggp,
            tc.tile_pool(name="tr", bufs=2, space="PSUM") as trp,
            tc.tile_pool(name="z", bufs=2, space="PSUM") as zp,
            tc.tile_pool(name="acc", bufs=1, space="PSUM") as accp,
        ):
            ld_sb = constp.tile([P, B], dt)
            nc.sync.dma_start(ld_sb[:], ld_t[:])
            val_sb = constp.tile([P, B], dt)
            nc.sync.dma_start(val_sb[:], val_t[:])
            s_sb = constp.tile([1, n_groups * P], f32)
            nc.sync.dma_start(s_sb[:], s_t[:])
            wt_sb = constp.tile([D, D], f32)
            nc.sync.dma_start(wt_sb[:], wt_t[:])
            brow_sb = constp.tile([1, D], f32)
            nc.sync.dma_start(brow_sb[:], brow_t[:])
            iota_sb = constp.tile([P, P], dt)
            nc.sync.dma_start(iota_sb[:], iota_t[:])
            ident_sb = constp.tile([P, P], f32)
            nc.sync.dma_start(ident_sb[:], ident_t[:])
            ones_sb = constp.tile([P, 1], f32)
            nc.sync.dma_start(ones_sb[:], ones_t[:])

            agg_acc = constp.tile([P, n_groups * P], f32)
            out_acc = accp.tile([1, D], f32)

            Copy = mybir.ActivationFunctionType.Copy
            Relu = mybir.ActivationFunctionType.Relu

            n_tails = [0]

            def tail(gg):
                gsl = slice(gg * P, (gg + 1) * P)
                aggT_ps = trp.tile([P, D], f32)
                nc.tensor.transpose(
                    out=aggT_ps[:], in_=agg_acc[:, gsl], identity=ident_sb[:]
                )
                aggT_sb = gsbp.tile([P, D], f32, tag="aggT_sb")
                nc.scalar.activation(out=aggT_sb[:], in_=aggT_ps[:], func=Copy)
                z_ps = zp.tile([P, D], f32)
                nc.tensor.matmul(
                    out=z_ps[:], lhsT=aggT_sb[:], rhs=wt_sb[:], start=True, stop=False
                )
                nc.tensor.matmul(
                    out=z_ps[:],
                    lhsT=s_sb[0:1, gsl],
                    rhs=brow_sb[:],
                    start=False,
                    stop=True,
                )
                relu_sb = gsbp.tile([P, D], f32, tag="relu")
                nc.scalar.activation(out=relu_sb[:], in_=z_ps[:], func=Relu)
                gi = n_tails[0]
                n_tails[0] += 1
                nc.tensor.matmul(
                    out=out_acc[0:1, :],
                    lhsT=ones_sb[:],
                    rhs=relu_sb[:],
                    start=(gi == 0),
                    stop=(gi == n_groups - 1),
                )

            oh_count = [0]

            def make_onehot(bb):
                oh = ohp.tile([P, P], dt)
                eng = nc.vector
                if oh_pool_frac > 0:
                    oh_count[0] += 1
                    if (oh_count[0] % 1000) < oh_pool_frac * 1000:
                        eng = nc.gpsimd
                eng.tensor_scalar(
                    oh[:],
                    iota_sb[:],
                    ld_sb[:, bb : bb + 1],
                    val_sb[:, bb : bb + 1],
                    mybir.AluOpType.is_equal,
                    mybir.AluOpType.mult,
                )
                return oh

            for si, st in enumerate(structure):
                NW = st["NW"]
                if NW == 0:
                    continue
                idx_sb = idxp.tile([P, NW * 8], mybir.dt.int16, tag="idx")
                nc.sync.dma_start(idx_sb[:], idx_ts[si][:])
                fwin = feature_t[st["row0"] : st["row0"] + st["rows"], :]
                msg = None
                wb_cursor = 0
                for gg, nbk in st["buckets"]:
                    agg_ps = aggp.tile([P, D], f32)
                    for j in range(nbk):
                        wb = wb_cursor + j
                        c, r = divmod(wb, CH)
                        if r == 0:
                            cw = min(CH, NW - c * CH)
                            msg = msgp.tile([P, CH, D], dt, tag="msg")
                            nc.gpsimd.dma_gather(
                                out_ap=msg[:, :cw, :],
                                in_ap=fwin,
                                idxs_ap=idx_sb[:, c * CH * 8 : (c * CH + cw) * 8],
                                num_idxs=cw * P,
                                num_idxs_reg=cw * P,
                                elem_size=D,
                            )
                        bb = st["woff"] + wb
                        oh = make_onehot(bb)
                        nc.tensor.matmul(
                            out=agg_ps[:],
                            lhsT=oh[:],
                            rhs=msg[:, r, :],
                            start=(j == 0),
                            stop=(j == nbk - 1),
                        )
                    wb_cursor += nbk
                    gsl = slice(gg * P, (gg + 1) * P)
                    if si == first_win_of_g[gg]:
                        nc.scalar.activation(
                            out=agg_acc[:, gsl], in_=agg_ps[:], func=Copy
                        )
                    else:
                        nc.vector.tensor_tensor(
                            out=agg_acc[:, gsl],
                            in0=agg_acc[:, gsl],
                            in1=agg_ps[:],
                            op=mybir.AluOpType.add,
                        )
                    if last_win_of_g[gg] == si:
                        tail(gg)

            # groups with no edges at all: agg is zero -> z = 0 -> relu 0.
            # still must contribute to the reduce chain count; memset + tail.
            for gg in range(n_groups):
                if gg not in seen_g:
                    nc.vector.memset(agg_acc[:, gg * P : (gg + 1) * P], 0.0)
                    tail(gg)

            res_sb = constp.tile([1, D], f32)
            nc.vector.tensor_copy(res_sb[:], out_acc[0:1, :])
            nc.sync.dma_start(out_t[:], res_sb[:])

    nc.compile()
    return nc


def kernel(feature, edge_src, edge_dst, edge_val, W, b):
    N, D = feature.shape
    E = edge_src.shape[0]
    assert D == P

    feature = np.ascontiguousarray(feature, dtype=np.float32)
    edge_src = np.asarray(edge_src, dtype=np.int64)
    edge_dst = np.asarray(edge_dst, dtype=np.int64)
    edge_val = np.asarray(edge_val, dtype=np.float32)

    plan = _plan(N, E, edge_src, edge_dst, edge_val)

    dt = mybir.dt.float32
    nc = _build_program(N, D, plan, dt)

    wt = np.ascontiguousarray(W.astype(np.float32).T)
    brow = np.ascontiguousarray(b.astype(np.float32).reshape(1, D))
    iota = np.tile(np.arange(P, dtype=np.float32), (P, 1))
    ident = np.eye(P, dtype=np.float32)
    ones = np.ones((P, 1), dtype=np.float32)

    in_maps = []
    for c in range(N_CORES):
        m = {
            "feature": feature,
            "ld": plan["ldv"][c],
            "val": plan["valv"][c],
            "s": plan["s"][c],
            "wt": wt,
            "brow": brow,
            "iota": iota,
            "ident": ident,
            "ones": ones,
        }
        for st in plan["structure"]:
            m[f"idxw{st['w']}"] = plan["idx"][st["w"]][c]
        in_maps.append(m)

    tkw = {}
    tdir = os.environ.get("GNN_TRACE_DIR")
    if tdir:
        os.makedirs(tdir, exist_ok=True)
        tkw["tmpdir"] = tdir
    res = run_bass_kernel_spmd(nc, in_maps, core_ids=list(range(N_CORES)), **tkw)
    global LAST_EXEC_NS
    LAST_EXEC_NS = res.exec_time_ns
    parts = np.stack([res.results[c]["out"][0] for c in range(N_CORES)])
    return (parts.sum(axis=0, dtype=np.float64) / N).astype(np.float32)



# revision 23
# speedup vs baseline: 1.0500x; 1.0500x over previous
"""GNN message-passing kernel for Trainium2 (8 NeuronCores, SPMD).

Computes out[D] = mean_n relu(segment_sum(val * (feat @ W.T + b)[src], dst))
reformulated (aggregate-then-transform, exact):
    aggT[d, n] = sum_{e: dst=n} val[e] * feature[src[e], d]
    zT[do, n]  = sum_d W.T[d, do] * aggT[d, n] + b[do] * s[n],  s = segsum(val)
    out        = sum_n relu(zT[:, n]) / N

Sharding: edges partitioned by destination node across 8 cores; each core owns
12500 destinations split into 98 groups of 128. Per 128-edge block, source
rows are fetched (bf16) and scattered into a PSUM aggT tile by a one-hot
matmul (lhsT=msg, rhs=onehot -> aggT directly, no transpose needed).

The row gather is split across two DMA mechanisms with different bottlenecks
so they overlap:
  - dma_gather (gpsimd ucode): ~8.4 ns/row Q7 descriptor emission, fast
    pipelined SDMA transfer. Gets only FULL 128-edge blocks (zero padding);
    int16 idx => 4 source windows of 32768 rows.
  - indirect_dma_start (stock SWDGE DGE): cheap emission but the aggregated
    packets serialize the random reads (~18 ns/row of shared SDMA time).
    Gets the per-(window,group) leftovers; int32 idx, no window limit.
Groups are processed in batches of 8 (8 concurrent PSUM aggT tiles); one-hots
are built 8 blocks at a time with broadcast-AP tensor_tensor ops on DVE.
"""

import math
import os
import sys

import numpy as np

LAST_EXEC_NS = None

for _p in ("/opt/trn_rl_repo",):
    if _p not in sys.path:
        sys.path.insert(0, _p)

import concourse.bacc as bacc
import concourse.bass as bass
import concourse.mybir as mybir
import concourse.tile as tile
from concourse.bass_utils import run_bass_kernel_spmd

P = 128
N_CORES = 8
WS = 32768          # dma_gather window rows (int16 idx reach)
GB = 4              # groups per batch (PSUM aggT tiles in flight)
KI = 32             # indirect-call width cap, blocks per call
MAXG = 8            # dma_gather blocks per call (1024 idx hard cap)
MIX_FRAC = 0.4      # fraction of (w,g) buckets whose gather share rounds UP


def _plan(N, E, edge_src, edge_dst, edge_val):
    """Host-side layout planning (numpy only). Returns per-core input arrays
    plus the compile-time block structure shared by all cores."""
    npc = (N + N_CORES - 1) // N_CORES          # nodes per core
    ng = (npc + P - 1) // P                     # groups per core
    nw = (N + WS - 1) // WS                     # gather windows

    core = edge_dst // npc
    local = edge_dst - core * npc
    g = local // P
    ld = local % P
    w = edge_src // WS

    # counts per (core, w, g)
    key_cwg = (core * nw + w) * ng + g
    cnt_cwg = np.bincount(key_cwg, minlength=N_CORES * nw * ng).reshape(
        N_CORES, nw, ng
    )
    # dma_gather blocks per (w, g): shared across cores. Base = floor of the
    # emptiest core's count; a MIX_FRAC subset rounds up one extra block
    # (padded with val=0 slots where a core runs short). This knob balances
    # gpsimd descriptor-emission time (gather) against SDMA serialized-read
    # time (indirect leftovers).
    mn = cnt_cwg.min(axis=0)                     # [nw, ng]
    nbg = mn // P
    bump = (np.arange(nw)[:, None] * ng + np.arange(ng)[None, :]) % 1000
    nbg = nbg + ((bump < MIX_FRAC * 1000) & (mn % P > 0)).astype(np.int64)
    nbg = nbg.astype(np.int64)

    # leftovers go to the indirect mechanism, pooled per group
    left_cg = (cnt_cwg - nbg[None] * P).clip(min=0).sum(axis=1)  # [NC, ng]
    nbi = ((left_cg.max(axis=0) + P - 1) // P).astype(np.int64)  # [ng]
    assert (nbi <= KI).all(), "indirect blocks per group exceed call cap"

    # ---- block id assignment (shared): group-batch-major processing order --
    # per batch: for each window, that batch's gather blocks (contiguous);
    # then the batch's indirect blocks (pooled leftovers per group).
    n_batches = (ng + GB - 1) // GB
    gcalls = []     # (batch, w, block_id_start, nblocks) per dma_gather call
    icalls = []     # (batch, block_id_start, nblocks) per indirect call
    gblk_of = {}    # (w, g, j) -> block id
    iblk_of = {}    # (g, j) -> block id
    bid = 0
    gather_slots = 0
    gslot_of_blk = {}           # gather block id -> slot offset in idx16
    for bi in range(n_batches):
        gs = range(bi * GB, min((bi + 1) * GB, ng))
        for wv in range(nw):
            call_blocks = []
            for gg in gs:
                for j in range(int(nbg[wv, gg])):
                    gblk_of[(wv, gg, j)] = bid
                    call_blocks.append(bid)
                    bid += 1
            # split into calls of <= MAXG blocks
            for s in range(0, len(call_blocks), MAXG):
                cb = call_blocks[s : s + MAXG]
                for jj, b in enumerate(cb):
                    gslot_of_blk[b] = gather_slots + jj * P
                gcalls.append((bi, wv, cb[0], len(cb)))
                gather_slots += len(cb) * P
        ib0 = bid
        for gg in gs:
            for j in range(int(nbi[gg])):
                iblk_of[(gg, j)] = bid
                bid += 1
        # indirect calls for this batch, chunks of <= KI blocks
        for s in range(ib0, bid, KI):
            icalls.append((bi, s, min(KI, bid - s)))
    B = bid
    n_gather_blocks = len(gblk_of)
    n_ind_blocks = len(iblk_of)

    # ---- per-edge slot assignment (per core) ------------------------------
    order = np.lexsort((edge_src, g, w, core))   # (core, w, g) buckets
    c_s = core[order]
    w_s = w[order]
    g_s = g[order]
    ld_s = ld[order]
    src_s = edge_src[order]
    val_s = edge_val[order]

    # rank of edge within its (core, w, g) bucket
    kk = (c_s * nw + w_s) * ng + g_s
    flat_cnt = np.bincount(kk, minlength=N_CORES * nw * ng)
    bucket_start = np.concatenate(([0], np.cumsum(flat_cnt)[:-1]))
    rank = np.arange(E, dtype=np.int64) - bucket_start[kk]

    cap = (nbg[w_s, g_s] * P).astype(np.int64)
    is_g = rank < cap

    ldv = np.zeros((N_CORES, P, B), np.float32)
    valv = np.zeros((N_CORES, P, B), np.float32)

    # gather-mech edges: slot = rank within (w,g) gather region
    mg = is_g
    blk_j = rank[mg] // P
    lane_g = rank[mg] % P
    base_ids = np.zeros((nw, ng), np.int64)
    for (wv, gg, j), b in gblk_of.items():
        if j == 0:
            base_ids[wv, gg] = b
    gbid = base_ids[w_s[mg], g_s[mg]] + blk_j
    ldv[c_s[mg], lane_g, gbid] = ld_s[mg].astype(np.float32)
    valv[c_s[mg], lane_g, gbid] = val_s[mg]

    # gather idx tensor: [NC, 128, gather_slots/16] int16 (16-ch wrap, x8)
    # slot position within the call: linear r -> lane r%128, call-block r//128;
    # idx channel = r%16, col = r//16 (matches dma_gather ucode convention)
    gcols = gather_slots // 16
    idx16 = np.zeros((N_CORES, 16, gcols), np.int16)
    slot_arr = np.zeros(B, np.int64)
    for b, so in gslot_of_blk.items():
        slot_arr[b] = so
    r_lin = slot_arr[gbid] + lane_g
    src_rel = (src_s[mg] - w_s[mg] * WS).astype(np.int16)
    idx16[c_s[mg], r_lin % 16, r_lin // 16] = src_rel
    idx16_full = np.ascontiguousarray(np.tile(idx16, (1, 8, 1)))

    # indirect-mech edges: pooled leftovers per (core, g)
    mi = ~is_g
    ci = c_s[mi]
    gi = g_s[mi]
    ki2 = ci * ng + gi
    icnt = np.bincount(ki2, minlength=N_CORES * ng)
    istart = np.concatenate(([0], np.cumsum(icnt)[:-1]))
    irank = np.arange(mi.sum(), dtype=np.int64) - istart[ki2]
    ibase = np.zeros(ng, np.int64)
    for (gg, j), b in iblk_of.items():
        if j == 0:
            ibase[gg] = b
    ibid = ibase[gi] + irank // P
    lane_i = irank % P
    ldv[ci, lane_i, ibid] = ld_s[mi].astype(np.float32)
    valv[ci, lane_i, ibid] = val_s[mi]

    # indirect idx tensor: [NC, 128, n_ind_cols] int32; for a call covering
    # block-cols [k0, k0+nk): descriptor j (partition-fastest consumption)
    # serves out[p, k] with j = p*nk + k; host stores src at
    # idx32[j % 128, colbase + j // 128].
    n_ind_cols = max(1, n_ind_blocks)
    idx32 = np.zeros((N_CORES, P, n_ind_cols), np.int32)
    nk_of_blk = np.zeros(B, np.int64)
    k_in_call_of_blk = np.zeros(B, np.int64)
    colbase_of_blk = np.zeros(B, np.int64)
    icolbase_of_call = {}
    col = 0
    for (bi, s0, nk) in icalls:
        icolbase_of_call[s0] = col
        for j in range(nk):
            nk_of_blk[s0 + j] = nk
            k_in_call_of_blk[s0 + j] = j
            colbase_of_blk[s0 + j] = col
        col += nk
    j_lin = lane_i * nk_of_blk[ibid] + k_in_call_of_blk[ibid]
    idx32[ci, j_lin % P, colbase_of_blk[ibid] + j_lin // P] = src_s[mi]

    # segment sums of val per destination
    skey = (core * ng + g) * P + ld
    s = np.bincount(
        skey, weights=edge_val.astype(np.float64), minlength=N_CORES * ng * P
    ).astype(np.float32).reshape(N_CORES, 1, ng * P)

    # per-group block lists in processing order
    blocks_of_g = {gg: [] for gg in range(ng)}
    for (wv, gg, j), b in sorted(gblk_of.items(), key=lambda kv: kv[1]):
        blocks_of_g[gg].append(b)
    for (gg, j), b in sorted(iblk_of.items(), key=lambda kv: kv[1]):
        blocks_of_g[gg].append(b)

    return {
        "npc": npc, "ng": ng, "nw": nw, "B": B,
        "n_batches": n_batches,
        "gcalls": gcalls, "icalls": icalls,
        "gslot_of_blk": gslot_of_blk,
        "icolbase_of_call": icolbase_of_call,
        "gather_slots": gather_slots,
        "n_ind_cols": n_ind_cols,
        "blocks_of_g": blocks_of_g,
        "ldv": ldv, "valv": valv,
        "idx16": idx16_full, "idx32": idx32, "s": s,
    }


def _build_program(N, D, plan):
    bf16 = mybir.dt.bfloat16
    f32 = mybir.dt.float32
    ng = plan["ng"]
    B = plan["B"]
    gather_slots = plan["gather_slots"]
    n_ind_cols = plan["n_ind_cols"]
    gcalls = plan["gcalls"]
    icalls = plan["icalls"]
    gslot_of_blk = plan["gslot_of_blk"]
    icolbase_of_call = plan["icolbase_of_call"]
    blocks_of_g = plan["blocks_of_g"]
    n_batches = plan["n_batches"]

    from collections import Counter

    gc_per_batch = Counter(bi for (bi, _w, _b, _n) in gcalls)
    ic_per_batch = Counter(bi for (bi, _s, _n) in icalls)
    gmsg_bufs = 2 * max(gc_per_batch.values(), default=1) + 2
    imsg_bufs = 2 * max(ic_per_batch.values(), default=1) + 1
    max_nk = max((nk for (_b, _s, nk) in icalls), default=1)

    nc = bacc.Bacc("TRN2", target_bir_lowering=False, debug=False,
                   num_devices=N_CORES)

    feature_t = nc.dram_tensor("feature", [N, D], bf16, kind="ExternalInput")
    ld_t = nc.dram_tensor("ld", [P, B], bf16, kind="ExternalInput")
    val_t = nc.dram_tensor("val", [P, B], bf16, kind="ExternalInput")
    s_t = nc.dram_tensor("s", [1, ng * P], bf16, kind="ExternalInput")
    wt_t = nc.dram_tensor("wt", [D, D], bf16, kind="ExternalInput")
    brow_t = nc.dram_tensor("brow", [1, D], bf16, kind="ExternalInput")
    iota_t = nc.dram_tensor("iota", [P, GB * P], bf16, kind="ExternalInput")
    idx16_t = nc.dram_tensor("idx16", [P, gather_slots // 16],
                             mybir.dt.int16, kind="ExternalInput")
    idx32_t = nc.dram_tensor("idx32", [P, n_ind_cols], mybir.dt.int32,
                             kind="ExternalInput")
    out_t = nc.dram_tensor("out", [P, 1], f32, kind="ExternalOutput")

    Copy = mybir.ActivationFunctionType.Copy
    Relu = mybir.ActivationFunctionType.Relu

    with tile.TileContext(nc) as tc:
        with (
            tc.tile_pool(name="const", bufs=1) as constp,
            tc.tile_pool(name="gmsg", bufs=gmsg_bufs) as gmsgp,
            tc.tile_pool(name="imsg", bufs=imsg_bufs) as imsgp,
            tc.tile_pool(name="oh", bufs=2 * GB + 2) as ohp,
            tc.tile_pool(name="tailsb", bufs=4) as tailp,
            tc.tile_pool(name="agg", bufs=5, space="PSUM") as aggp,
            tc.tile_pool(name="z", bufs=2, space="PSUM") as zp,
        ):
            ld_sb = constp.tile([P, B], bf16)
            nc.sync.dma_start(ld_sb[:], ld_t[:])
            val_sb = constp.tile([P, B], bf16)
            nc.sync.dma_start(val_sb[:], val_t[:])
            s_sb = constp.tile([1, ng * P], bf16)
            nc.sync.dma_start(s_sb[:], s_t[:])
            wt_sb = constp.tile([D, D], bf16)
            nc.sync.dma_start(wt_sb[:], wt_t[:])
            brow_sb = constp.tile([1, D], bf16)
            nc.sync.dma_start(brow_sb[:], brow_t[:])
            iota_sb = constp.tile([P, GB * P], bf16)
            nc.sync.dma_start(iota_sb[:], iota_t[:])
            idx16_sb = constp.tile([P, gather_slots // 16], mybir.dt.int16)
            nc.sync.dma_start(idx16_sb[:], idx16_t[:])
            idx32_sb = constp.tile([P, n_ind_cols], mybir.dt.int32)
            nc.sync.dma_start(idx32_sb[:], idx32_t[:])

            rcols = constp.tile([P, ng], f32)
            nc.vector.memset(rcols[:], 0.0)

            gcalls_of_batch = {}
            for (bi, wv, b0, nb) in gcalls:
                gcalls_of_batch.setdefault(bi, []).append((wv, b0, nb))
            icalls_of_batch = {}
            for (bi, s0, nk) in icalls:
                icalls_of_batch.setdefault(bi, []).append((s0, nk))

            def build_oh(blocks):
                """One-hot strip(s) for consecutive-id runs among blocks."""
                out = {}
                runs = []
                run = [blocks[0]]
                for b in blocks[1:]:
                    if b == run[-1] + 1 and len(run) < GB:
                        run.append(b)
                    else:
                        runs.append(run)
                        run = [b]
                runs.append(run)
                for run in runs:
                    n = len(run)
                    b0 = run[0]
                    oh = ohp.tile([P, GB, P], bf16, tag="oh")
                    ldb = ld_sb[:, b0 : b0 + n].unsqueeze(2).to_broadcast(
                        [P, n, P]
                    )
                    valb = val_sb[:, b0 : b0 + n].unsqueeze(2).to_broadcast(
                        [P, n, P]
                    )
                    nc.vector.tensor_tensor(
                        out=oh[:, :n, :],
                        in0=iota_sb[:, : n * P].rearrange(
                            "p (n q) -> p n q", n=n
                        ),
                        in1=ldb, op=mybir.AluOpType.is_equal,
                    )
                    nc.vector.tensor_tensor(
                        out=oh[:, :n, :], in0=oh[:, :n, :], in1=valb,
                        op=mybir.AluOpType.mult,
                    )
                    for j in range(n):
                        out[b0 + j] = (oh, j)
                return out

            # ---- per batch: msg DMAs, one-hot builds, matmuls, tails -------
            for bi in range(n_batches):
                msg_of_blk = {}
                for (wv, b0, nb) in gcalls_of_batch.get(bi, []):
                    t = gmsgp.tile([P, MAXG, D], bf16, tag="gmsg")
                    off = gslot_of_blk[b0]
                    fwin = feature_t[wv * WS : min((wv + 1) * WS, N), :]
                    nc.gpsimd.dma_gather(
                        out_ap=t[:, :nb, :],
                        in_ap=fwin,
                        idxs_ap=idx16_sb[:, off // 16 : (off + nb * P) // 16],
                        num_idxs=nb * P,
                        num_idxs_reg=nb * P,
                        elem_size=D,
                    )
                    for j in range(nb):
                        msg_of_blk[b0 + j] = (t, j)
                for (s0, nk) in icalls_of_batch.get(bi, []):
                    t = imsgp.tile([P, max_nk, D], bf16, tag="imsg")
                    colbase = icolbase_of_call[s0]
                    nc.gpsimd.indirect_dma_start(
                        out=t[:, :nk, :],
                        out_offset=None,
                        in_=feature_t[:, :],
                        in_offset=bass.IndirectOffsetOnAxis(
                            ap=idx32_sb[:, colbase : colbase + nk], axis=0
                        ),
                    )
                    for j in range(nk):
                        msg_of_blk[s0 + j] = (t, j)

                batch_groups = range(bi * GB, min((bi + 1) * GB, ng))
                for gg in batch_groups:
                    blks = blocks_of_g[gg]
                    if not blks:
                        continue
                    oh_of_blk = build_oh(blks)
                    agg_ps = aggp.tile([P, P], f32)
                    for i, b in enumerate(blks):
                        mt, mj = msg_of_blk[b]
                        ot, oj = oh_of_blk[b]
                        nc.tensor.matmul(
                            out=agg_ps[:],
                            lhsT=mt[:, mj, :],
                            rhs=ot[:, oj, :],
                            start=(i == 0),
                            stop=(i == len(blks) - 1),
                        )
                    aggT_sb = tailp.tile([P, P], bf16, tag="aggT")
                    nc.scalar.activation(
                        out=aggT_sb[:], in_=agg_ps[:], func=Copy
                    )
                    z_ps = zp.tile([P, P], f32)
                    nc.tensor.matmul(
                        out=z_ps[:], lhsT=wt_sb[:], rhs=aggT_sb[:],
                        start=True, stop=False,
                    )
                    gsl = slice(gg * P, (gg + 1) * P)
                    nc.tensor.matmul(
                        out=z_ps[:], lhsT=brow_sb[:], rhs=s_sb[0:1, gsl],
                        start=False, stop=True,
                    )
                    relu_sb = tailp.tile([P, P], bf16, tag="relu")
                    nc.scalar.activation(
                        out=relu_sb[:], in_=z_ps[:], func=Relu
                    )
                    nc.vector.tensor_reduce(
                        out=rcols[:, gg : gg + 1], in_=relu_sb[:],
                        axis=mybir.AxisListType.X, op=mybir.AluOpType.add,
                    )

            out_sb = constp.tile([P, 1], f32)
            nc.vector.tensor_reduce(
                out=out_sb[:], in_=rcols[:],
                axis=mybir.AxisListType.X, op=mybir.AluOpType.add,
            )
            nc.sync.dma_start(out_t[:], out_sb[:])

    nc.compile()
    return nc


def kernel(feature, edge_src, edge_dst, edge_val, W, b):
    import ml_dtypes

    N, D = feature.shape
    E = edge_src.shape[0]
    assert D == P

    feature = np.ascontiguousarray(feature, dtype=np.float32)
    edge_src = np.asarray(edge_src, dtype=np.int64)
    edge_dst = np.asarray(edge_dst, dtype=np.int64)
    edge_val = np.asarray(edge_val, dtype=np.float32)

    plan = _plan(N, E, edge_src, edge_dst, edge_val)
    nc = _build_program(N, D, plan)

    bf = ml_dtypes.bfloat16
    feat_bf = feature.astype(bf)
    wt = np.ascontiguousarray(W.astype(np.float32).T).astype(bf)
    brow = b.astype(np.float32).reshape(1, D).astype(bf)
    iota = np.tile(np.arange(P, dtype=np.float32), (P, GB)).astype(bf)

    in_maps = []
    for c in range(N_CORES):
        in_maps.append({
            "feature": feat_bf,
            "ld": plan["ldv"][c].astype(bf),
            "val": plan["valv"][c].astype(bf),
            "s": plan["s"][c].astype(bf),
            "wt": wt,
            "brow": brow,
            "iota": iota,
            "idx16": plan["idx16"][c],
            "idx32": plan["idx32"][c],
        })

    tkw = {}
    tdir = os.environ.get("GNN_TRACE_DIR")
    if tdir:
        os.makedirs(tdir, exist_ok=True)
        tkw["tmpdir"] = tdir
    res = run_bass_kernel_spmd(nc, in_maps, core_ids=list(range(N_CORES)), **tkw)
    global LAST_EXEC_NS
    LAST_EXEC_NS = res.exec_time_ns
    parts = np.stack(
        [res.results[c]["out"][:, 0].astype(np.float64) for c in range(N_CORES)]
    )
    return (parts.sum(axis=0) / N).astype(np.float32)
